# revision 5
# baseline (speedup 1.0000x reference)
"""Fused multi-head bilinear attention (softmax over query axis m) on 8 trn2 cores.

Reference computation (b=2, m=n=2048, e=128, k=8, d=16):
    r   = einsum('bmp,kpd->bmkd', x, lambda1) + bias_lambda
    A   = einsum('bmkd,kqd,bnq->kbmn', r, lambda2, y) * d**-0.5
    att = softmax(A, axis=m)
    r2  = einsum('kbmn,bmp,kpd->bnkd', att, x, theta1) + bias_theta
    out = einsum('bnkd,kqd->bnq', r2, theta2)

Sharding: 8 cores = 2 batches x 4 n-quarters (512 wide). Each core computes all 8
heads for its output slice out[b, nq*512:(nq+1)*512, :]; unshard is pure concat.

Per-core pipeline (all heads):
  X^T, Y^T via PE transposes; R^T = (X@L1)^T and S^T = (Y@L2)^T with heads packed
  into 32-partition strips (16 used + 16 zero) so K=16 matmuls stay 32-aligned;
  T = X@theta1 stored [m, (head,17)] with a ones column per head.
  Per head: A tiles [m128, 512] = R^T.T @ S^T (f32r), exp on ScalarE straight out
  of PSUM in 2048/1024-wide calls, then U^T[17, n] += [T|1]^T @ expA accumulated
  over m -- row 16 is the softmax denominator for free.  U^T is transposed back,
  normalized per-partition, stacked into r2[n, (k d)], and contracted with
  theta2^T over all 128 (k,d) at full PE width.
"""

import sys

sys.path.insert(0, "/opt/trn_rl_repo")

from contextlib import ExitStack

import numpy as np

import concourse.bass as bass
import concourse.tile as tile
from concourse import bacc, mybir
from concourse.bass import ds, ts
from concourse.masks import make_identity

F32 = mybir.dt.float32
F32R = mybir.dt.float32r
EXP = mybir.ActivationFunctionType.Exp

B, M, N, E, K, D = 2, 2048, 2048, 128, 8, 16
NCORES = 8
NSLICE = N // 4          # n columns per core (one batch, quarter of n)
MT = M // 128            # 16 m-tiles
SCALE = float(D) ** -0.5
# m-tile groups for A/exp staging: (start, len) in units of 512-wide m-tiles.
# len-4 groups use the 4-bank PSUM pool, len-2 the 2-bank pool.
GROUPS = [(0, 4), (4, 2), (6, 4), (10, 2), (12, 4)]


def _emit(tc: tile.TileContext, ctx: ExitStack, io: dict):
    nc = tc.nc
    xb, ybs, l1g, l2g, t1a, t2t, blg, btb, outb = (
        io["xb"], io["ybs"], io["l1g"], io["l2g"], io["t1a"], io["t2t"],
        io["blg"], io["btb"], io["outb"],
    )

    const = ctx.enter_context(tc.tile_pool(name="const", bufs=1))
    persist = ctx.enter_context(tc.tile_pool(name="persist", bufs=1))
    xin_pool = ctx.enter_context(tc.tile_pool(name="xin", bufs=4))
    expa4_pool = ctx.enter_context(tc.tile_pool(name="expa4", bufs=2))
    expa2_pool = ctx.enter_context(tc.tile_pool(name="expa2", bufs=2))
    usb_pool = ctx.enter_context(tc.tile_pool(name="usb", bufs=2))
    den_pool = ctx.enter_context(tc.tile_pool(name="den", bufs=4))
    r2t_pool = ctx.enter_context(tc.tile_pool(name="r2t", bufs=2))
    out_pool = ctx.enter_context(tc.tile_pool(name="outp", bufs=2))
    ps_a4 = ctx.enter_context(tc.tile_pool(name="ps_a4", bufs=1, space="PSUM"))
    ps_a2 = ctx.enter_context(tc.tile_pool(name="ps_a2", bufs=1, space="PSUM"))
    ps_u = ctx.enter_context(tc.tile_pool(name="ps_u", bufs=2, space="PSUM"))

    ident = const.tile([128, 128], F32)
    make_identity(nc, ident[:])

    # ---- parameter loads -------------------------------------------------
    L1 = const.tile([128, 2, 128], F32)   # strip-packed lambda1 per head-group
    L2 = const.tile([128, 2, 128], F32)
    T1A = const.tile([128, 128], F32)     # theta1 packed (k d)
    T2T = const.tile([128, 128], F32)     # theta2^T [(k d), q]
    BLG = const.tile([128, 2], F32)       # strip-packed bias_lambda
    BTB = const.tile([128, 128], F32)     # bias_theta broadcast over partitions
    for g in range(2):
        nc.sync.dma_start(L1[:, g, :], l1g[g])
        nc.sync.dma_start(L2[:, g, :], l2g[g])
    nc.sync.dma_start(T1A[:], t1a)
    nc.sync.dma_start(T2T[:], t2t)
    nc.sync.dma_start(BLG[:], blg)
    nc.sync.dma_start(BTB[:], btb)

    # ---- X^T / Y^T via PE transposes ------------------------------------
    XT = persist.tile([128, M], F32)      # [e, m]
    YT = persist.tile([128, NSLICE], F32)  # [e, n]
    for mt in range(MT):
        xin = xin_pool.tile([128, 128], F32, tag="xin")
        nc.sync.dma_start(xin[:], xb[ds(mt * 128, 128), :])
        pst = (ps_a4 if mt % 2 == 0 else ps_a2).tile(
            [128, 128], F32, tag="a4" if mt % 2 == 0 else "a2")
        nc.tensor.transpose(pst[:], xin[:], ident[:])
        nc.vector.tensor_copy(XT[:, ts(mt, 128)], pst[:])
    for nt in range(NSLICE // 128):
        yin = xin_pool.tile([128, 128], F32, tag="xin")
        nc.sync.dma_start(yin[:], ybs[ds(nt * 128, 128), :])
        pst = (ps_a4 if nt % 2 == 0 else ps_a2).tile(
            [128, 128], F32, tag="a4" if nt % 2 == 0 else "a2")
        nc.tensor.transpose(pst[:], yin[:], ident[:])
        nc.vector.tensor_copy(YT[:, ts(nt, 128)], pst[:])

    # ---- projections -----------------------------------------------------
    RT = persist.tile([128, 2, M], F32R)       # R^T strips [32h+j, g, m]
    ST = persist.tile([128, 2, NSLICE], F32R)  # S^T strips
    for g in range(2):
        for c in range(M // 512):
            ps = (ps_a4 if c % 2 == 0 else ps_a2).tile(
                [128, 512], F32, tag="a4" if c % 2 == 0 else "a2")
            nc.tensor.matmul(ps[:], lhsT=L1[:, g, :], rhs=XT[:, ts(c, 512)],
                             start=True, stop=True)
            nc.vector.tensor_scalar_add(RT[:, g, ts(c, 512)], ps[:], BLG[:, g:g + 1])
        ps = (ps_a4 if g == 0 else ps_a2).tile(
            [128, NSLICE], F32, tag="a4" if g == 0 else "a2")
        nc.tensor.matmul(ps[:], lhsT=L2[:, g, :], rhs=YT[:], start=True, stop=True)
        nc.vector.tensor_copy(ST[:, g, :], ps[:])

    # T_aug[m, (k, 17)]: per head 16 cols of X@theta1 plus a ones column.
    TAUG = persist.tile([128, MT, K * 17], F32R)
    ONES = const.tile([128, MT * K], F32)
    nc.gpsimd.memset(ONES[:], 1.0)
    nc.vector.tensor_copy(
        TAUG[:].rearrange("p mt (k s) -> p mt k s", k=K)[:, :, :, 16:17],
        ONES[:].rearrange("p (mt k) -> p mt k", k=K)[:, :, :, None])
    for mt in range(MT):
        ps = (ps_a4 if mt % 2 == 0 else ps_a2).tile(
            [128, 128], F32, tag="a4" if mt % 2 == 0 else "a2")
        nc.tensor.matmul(ps[:], lhsT=XT[:, ts(mt, 128)], rhs=T1A[:],
                         start=True, stop=True)
        nc.vector.tensor_copy(
            TAUG[:, mt, :].rearrange("p (k s) -> p k s", k=K)[:, :, 0:16],
            ps[:].rearrange("p (k d) -> p k d", k=K))

    # ---- main loop over heads -------------------------------------------
    R2 = persist.tile([128, NSLICE // 128, 128], F32)  # [n128, chunk, (k d)]
    for k in range(K):
        g, h = divmod(k, 4)
        strip = 32 * h
        U = ps_u.tile([17, NSLICE], F32, tag="u")
        for (mst, glen) in GROUPS:
            pool, tag = (ps_a4, "a4") if glen == 4 else (ps_a2, "a2")
            aps = pool.tile([128, 512 * glen], F32, tag=tag)
            for j in range(glen):
                mt = mst + j
                nc.tensor.matmul(
                    aps[:, ts(j, 512)],
                    lhsT=RT[strip:strip + 16, g, ds(mt * 128, 128)],
                    rhs=ST[strip:strip + 16, g, :],
                    start=True, stop=True, tile_position=(strip, 0))
            epool = expa4_pool if glen == 4 else expa2_pool
            expa = epool.tile([128, 512 * glen], F32R, tag="e%d" % glen)
            nc.scalar.activation(expa[:], aps[:], EXP, scale=SCALE)
            for j in range(glen):
                mt = mst + j
                nc.tensor.matmul(
                    U[:], lhsT=TAUG[:, mt, ds(17 * k, 17)],
                    rhs=expa[:, ts(j, 512)],
                    start=(mt == 0), stop=(mt == MT - 1))
        usb = usb_pool.tile([17, NSLICE], F32, tag="usb")
        nc.vector.tensor_copy(usb[:], U[:])
        for ch in range(NSLICE // 128):
            tp = ps_u.tile([128, 32], F32, tag="u")
            nc.tensor.transpose(tp[:, 0:17], usb[:, ts(ch, 128)], ident[0:17, 0:17])
            den = den_pool.tile([128, 1], F32, tag="den")
            nc.vector.reciprocal(den[:], tp[:, 16:17])
            nc.vector.tensor_scalar_mul(
                R2[:, ch, ds(16 * k, 16)], tp[:, 0:16], den[:])

    # ---- output: out[n, q] = (r2 + bias_theta) @ theta2^T ---------------
    for ch in range(NSLICE // 128):
        nc.vector.tensor_add(R2[:, ch, :], R2[:, ch, :], BTB[:])
        rp = ps_a2.tile([128, 128], F32, tag="a2")
        nc.tensor.transpose(rp[:], R2[:, ch, :], ident[:])
        r2t = r2t_pool.tile([128, 128], F32, tag="r2t")
        nc.vector.tensor_copy(r2t[:], rp[:])
        op = ps_a4.tile([128, 128], F32, tag="a4")
        nc.tensor.matmul(op[:], lhsT=r2t[:], rhs=T2T[:], start=True, stop=True)
        ob = out_pool.tile([128, 128], F32, tag="ob")
        nc.vector.tensor_copy(ob[:], op[:])
        nc.sync.dma_start(outb[ds(ch * 128, 128), :], ob[:])


_CACHE = {}


def build():
    if "nc" in _CACHE:
        return _CACHE["nc"]
    nc = bacc.Bacc("TRN2", target_bir_lowering=False, debug=False,
                   num_devices=NCORES)
    io = {
        "xb": nc.dram_tensor("xb", [M, E], F32, kind="ExternalInput").ap(),
        "ybs": nc.dram_tensor("ybs", [NSLICE, E], F32, kind="ExternalInput").ap(),
        "l1g": nc.dram_tensor("l1g", [2, E, 128], F32, kind="ExternalInput").ap(),
        "l2g": nc.dram_tensor("l2g", [2, E, 128], F32, kind="ExternalInput").ap(),
        "t1a": nc.dram_tensor("t1a", [E, 128], F32, kind="ExternalInput").ap(),
        "t2t": nc.dram_tensor("t2t", [128, E], F32, kind="ExternalInput").ap(),
        "blg": nc.dram_tensor("blg", [128, 2], F32, kind="ExternalInput").ap(),
        "btb": nc.dram_tensor("btb", [128, 128], F32, kind="ExternalInput").ap(),
        "outb": nc.dram_tensor("outb", [NSLICE, E], F32, kind="ExternalOutput").ap(),
    }
    with tile.TileContext(nc) as tc:
        with ExitStack() as ctx:
            _emit(tc, ctx, io)
    nc.compile()
    _CACHE["nc"] = nc
    return nc


def make_in_maps(x, y, lambda1, lambda2, theta1, theta2, bias_lambda, bias_theta):
    f = np.float32
    l1g = np.zeros((2, E, 128), f)
    l2g = np.zeros((2, E, 128), f)
    blg = np.zeros((128, 2), f)
    for g in range(2):
        for h in range(4):
            l1g[g, :, 32 * h:32 * h + 16] = lambda1[4 * g + h]
            l2g[g, :, 32 * h:32 * h + 16] = lambda2[4 * g + h]
            blg[32 * h:32 * h + 16, g] = bias_lambda[4 * g + h]
    t1a = np.ascontiguousarray(theta1.transpose(1, 0, 2).reshape(E, K * D))
    t2t = np.ascontiguousarray(theta2.transpose(0, 2, 1).reshape(K * D, E))
    btb = np.tile(bias_theta.reshape(1, K * D), (128, 1)).astype(f)
    maps = []
    for c in range(NCORES):
        b, q = divmod(c, 4)
        maps.append({
            "xb": np.ascontiguousarray(x[b], dtype=f),
            "ybs": np.ascontiguousarray(y[b, q * NSLICE:(q + 1) * NSLICE], dtype=f),
            "l1g": l1g, "l2g": l2g, "t1a": t1a, "t2t": t2t,
            "blg": blg, "btb": btb,
        })
    return maps


def kernel(x, y, lambda1, lambda2, theta1, theta2, bias_lambda, bias_theta):
    from concourse.bass_utils import run_bass_kernel_spmd
    nc = build()
    maps = make_in_maps(x, y, lambda1, lambda2, theta1, theta2,
                        bias_lambda, bias_theta)
    res = run_bass_kernel_spmd(nc, maps, list(range(NCORES)))
    out = np.empty((B, N, E), np.float32)
    for c in range(NCORES):
        b, q = divmod(c, 4)
        out[b, q * NSLICE:(q + 1) * NSLICE] = res.results[c]["outb"]
    return out
